# revision 1
# baseline (speedup 1.0000x reference)
"""Distributed single-head attention block for trn2 (8 NeuronCores).

reference:
    q = x @ Wq.T + bq ; k = x @ Wk.T + bk ; v = x @ Wv.T + bv
    out = x + softmax(q @ k.T / sqrt(D)) @ v       x: [4, 2048, 1024]

Sharding: 8 cores = 4 batches x 2 query-halves. Core c owns batch c//2 and
query rows [h*1024, (h+1)*1024) with h = c%2. Each core recomputes K/V for
its whole batch (duplicated across the pair; no collectives needed).

Device-side layouts (host pre-transposes + bf16-casts so the contraction
dim always lands on SBUF partitions):
    xT  [D, S]   bf16   x[b].T            -> K/V projections
    xqT [D, SQ]  bf16   x[b, half].T      -> Q projection
    xq  [SQ, D]  f32    x[b, half]        -> residual add
    w*T [D, D]   bf16   W.T
Projections emit qT/kT [e, s] (scores contraction over e) and v [s, e]
(attn contraction over keys). Softmax rows live on partitions: exp on
ScalarE with accum_out giving row sums for free; no max subtraction
(scores are O(10) for this model so exp cannot overflow in f32).
P is transposed 128x128 on TensorE (identity matmul) for the attn matmul.
"""

import numpy as np

B, S, D = 4, 2048, 1024
SQ = S // 2  # queries per core
NCORES = 8
DC = D // 128  # contraction chunks
EC = D // 128  # embed chunks
SC = S // 128  # key chunks
QT = SQ // 128  # query tiles per core

_cache = {}


def _build():
    import concourse.bass as bass
    import concourse.tile as tile
    from concourse import bacc, mybir
    from concourse.masks import make_identity

    f32 = mybir.dt.float32
    bf16 = mybir.dt.bfloat16
    Alu = mybir.AluOpType
    Act = mybir.ActivationFunctionType

    nc = bacc.Bacc(None, target_bir_lowering=False, debug=False)

    xT_d = nc.declare_dram_parameter("xT", [D, S], bf16, isOutput=False)
    xqT_d = nc.declare_dram_parameter("xqT", [D, SQ], bf16, isOutput=False)
    xq_d = nc.declare_dram_parameter("xq", [SQ, D], f32, isOutput=False)
    wq_d = nc.declare_dram_parameter("wqT", [D, D], bf16, isOutput=False)
    wk_d = nc.declare_dram_parameter("wkT", [D, D], bf16, isOutput=False)
    wv_d = nc.declare_dram_parameter("wvT", [D, D], bf16, isOutput=False)
    bq_d = nc.declare_dram_parameter("bq", [D], f32, isOutput=False)
    out_d = nc.declare_dram_parameter("out", [SQ, D], f32, isOutput=True)

    # V is projected only for this core's own key-half; the pair exchanges
    # halves via a pairwise AllGather. The AG slot order (rank0, rank1)
    # equals natural batch order, identical on both cores, so the readback
    # is uniform across the SPMD graph.
    vx_in = nc.dram_tensor("vx_in", [SC // 2, 128, D], bf16)
    vx_out = nc.dram_tensor("vx_out", [2, SC // 2, 128, D], bf16)

    with tile.TileContext(nc) as tc:
        with tc.tile_pool(name="pers", bufs=1) as pers:
            qT_sb = pers.tile([128, EC, SQ], bf16, tag="qT")
            kT_sb = pers.tile([128, EC, S], bf16, tag="kT")
            v_sb = pers.tile([128, SC, D], bf16, tag="v")
            ident = pers.tile([128, 128], bf16, tag="ident")
            make_identity(nc, ident)
            # Only the Q bias is applied on-device: the K bias cancels in
            # softmax (per-row constant), and the V bias is folded into the
            # residual input on the host (attention weights sum to 1).
            bq_sb = pers.tile([128, EC], f32, tag="bq")

            # PE warmup: dense dummy matmuls while the first input DMAs land,
            # so the HAM clock gate is already at 2.4GHz when real work starts.
            warm_sb = pers.tile([128, 512], bf16, tag="warm")
            warm_dump = pers.tile([128, 512], f32, tag="warm_dump")
            nc.vector.memset(warm_sb, 0.0)
            with tc.tile_pool(name="warm_ps", bufs=1, space="PSUM") as warm_ps:
                wps = warm_ps.tile([128, 512], f32, tag="wps")
                NWARM = 14
                for i in range(NWARM):
                    nc.tensor.matmul(
                        wps,
                        lhsT=warm_sb[:, 0:128],
                        rhs=warm_sb,
                        start=(i == 0),
                        stop=(i == NWARM - 1),
                    )
                nc.vector.tensor_copy(out=warm_dump, in_=wps)

            bq_ap = bq_d.ap()
            nc.scalar.dma_start(
                out=bq_sb,
                in_=bass.AP(tensor=bq_ap.tensor, offset=0, ap=[[1, 128], [128, EC]]),
            )

            with (
                tc.tile_pool(name="ld", bufs=1) as ld,
                tc.tile_pool(name="proj_ps", bufs=4, space="PSUM") as proj_ps,
            ):
                xT_sb = ld.tile([128, DC, S], bf16, tag="xT")
                xqT_sb = ld.tile([128, DC, SQ], bf16, tag="xqT")
                wq_sb = ld.tile([128, DC, D], bf16, tag="wq")
                wk_sb = ld.tile([128, DC, D], bf16, tag="wk")
                wv_sb = ld.tile([128, DC, D], bf16, tag="wv")
                # DMA priority order: v-own projection inputs first (they gate
                # the first matmuls AND the pairwise exchange), then k inputs,
                # then q weights.
                for dc in range(DC):
                    r = slice(dc * 128, (dc + 1) * 128)
                    nc.sync.dma_start(out=xqT_sb[:, dc, :], in_=xqT_d[r, :])
                    nc.sync.dma_start(out=wv_sb[:, dc, :], in_=wv_d[r, :])
                for dc in range(DC):
                    r = slice(dc * 128, (dc + 1) * 128)
                    nc.sync.dma_start(out=xT_sb[:, dc, :], in_=xT_d[r, :])
                    nc.sync.dma_start(out=wk_sb[:, dc, :], in_=wk_d[r, :])
                for dc in range(DC):
                    r = slice(dc * 128, (dc + 1) * 128)
                    nc.sync.dma_start(out=wq_sb[:, dc, :], in_=wq_d[r, :])

                # v-own [sk_own, e] for this core's key half, staged to DRAM
                # for the pairwise exchange
                with tc.tile_pool(name="vstage", bufs=8) as vstage:
                    for sc in range(SC // 2):
                        vt = vstage.tile([128, D], bf16, tag="vt")
                        for j in range(D // 512):
                            ps = proj_ps.tile([128, 512], f32, tag="ps")
                            for dc in range(DC):
                                nc.tensor.matmul(
                                    ps,
                                    lhsT=xqT_sb[:, dc, sc * 128 : (sc + 1) * 128],
                                    rhs=wv_sb[:, dc, j * 512 : (j + 1) * 512],
                                    start=(dc == 0),
                                    stop=(dc == DC - 1),
                                )
                            nc.vector.tensor_copy(
                                out=vt[:, j * 512 : (j + 1) * 512],
                                in_=ps,
                            )
                        nc.scalar.dma_start(out=vx_in[sc], in_=vt)
                    nc.gpsimd.collective_compute(
                        "AllGather",
                        mybir.AluOpType.bypass,
                        replica_groups=[[0, 1], [2, 3], [4, 5], [6, 7]],
                        ins=[vx_in.ap().opt()],
                        outs=[vx_out.ap().opt()],
                    )
                    for r2 in range(2):
                        for sc in range(SC // 2):
                            nc.scalar.dma_start(
                                out=v_sb[:, r2 * (SC // 2) + sc, :],
                                in_=vx_out[r2, sc],
                            )

                # kT[e, sk] over the full batch
                for ec in range(EC):
                    for j in range(S // 512):
                        ps = proj_ps.tile([128, 512], f32, tag="ps")
                        for dc in range(DC):
                            nc.tensor.matmul(
                                ps,
                                lhsT=wk_sb[:, dc, ec * 128 : (ec + 1) * 128],
                                rhs=xT_sb[:, dc, j * 512 : (j + 1) * 512],
                                start=(dc == 0),
                                stop=(dc == DC - 1),
                            )
                        nc.vector.tensor_copy(
                            out=kT_sb[:, ec, j * 512 : (j + 1) * 512],
                            in_=ps,
                        )
                # qT[e, sq] = sum_d wqT[d, e] * xqT[d, sq]  (+bq per-partition)
                for j in range(SQ // 512):
                    for ec in range(EC):
                        ps = proj_ps.tile([128, 512], f32, tag="ps")
                        for dc in range(DC):
                            nc.tensor.matmul(
                                ps,
                                lhsT=wq_sb[:, dc, ec * 128 : (ec + 1) * 128],
                                rhs=xqT_sb[:, dc, j * 512 : (j + 1) * 512],
                                start=(dc == 0),
                                stop=(dc == DC - 1),
                            )
                        nc.vector.tensor_scalar_add(
                            out=qT_sb[:, ec, j * 512 : (j + 1) * 512],
                            in0=ps,
                            scalar1=bq_sb[:, ec : ec + 1],
                        )

            with (
                tc.tile_pool(name="att", bufs=2) as att,
                tc.tile_pool(name="small", bufs=2) as small,
                tc.tile_pool(name="score_ps", bufs=3, space="PSUM") as score_ps,
                tc.tile_pool(name="tr_ps", bufs=2, space="PSUM") as tr_ps,
                tc.tile_pool(name="attn_ps", bufs=3, space="PSUM") as attn_ps,
            ):
                inv_sqrt_d = float(1.0 / np.sqrt(D))
                # pass 1: scores -> exp -> denominators -> P transposes for
                # ALL q-tiles first. The PE queue is in-order, so keeping
                # every attn matmul behind all score/transpose work makes the
                # kernel robust to a late V exchange (no head-of-line block).
                PT_list, recip_list = [], []
                for qt in range(QT):
                    qsl = slice(qt * 128, (qt + 1) * 128)
                    P_sb = att.tile([128, S], bf16, tag="P", bufs=3)
                    den4 = small.tile([128, S // 512], f32, tag="den4", bufs=4)
                    for kc in range(S // 512):
                        ps = score_ps.tile([128, 512], f32, tag="score")
                        for ec in range(EC):
                            nc.tensor.matmul(
                                ps,
                                lhsT=qT_sb[:, ec, qsl],
                                rhs=kT_sb[:, ec, kc * 512 : (kc + 1) * 512],
                                start=(ec == 0),
                                stop=(ec == EC - 1),
                            )
                        nc.scalar.activation(
                            out=P_sb[:, kc * 512 : (kc + 1) * 512],
                            in_=ps,
                            func=Act.Exp,
                            scale=inv_sqrt_d,
                            accum_out=den4[:, kc : kc + 1],
                        )
                    recip = small.tile([128, 1], f32, tag="recip", bufs=QT)
                    den = small.tile([128, 1], f32, tag="den", bufs=4)
                    nc.vector.reduce_sum(out=den, in_=den4, axis=mybir.AxisListType.X)
                    nc.vector.reciprocal(recip, den)

                    PT_sb = att.tile([128, SC, 128], bf16, tag="PT", bufs=QT)
                    for j in range(SC):
                        tp = tr_ps.tile([128, 128], bf16, tag="tr")
                        nc.tensor.transpose(tp, P_sb[:, j * 128 : (j + 1) * 128], ident)
                        nc.vector.tensor_copy(out=PT_sb[:, j, :], in_=tp)
                    PT_list.append(PT_sb)
                    recip_list.append(recip)

                # pass 2: attn + epilogue per q-tile (needs the V exchange)
                for qt in range(QT):
                    qsl = slice(qt * 128, (qt + 1) * 128)
                    PT_sb = PT_list[qt]
                    recip = recip_list[qt]
                    xq_sb = att.tile([128, D], f32, tag="xq", bufs=3)
                    nc.sync.dma_start(out=xq_sb, in_=xq_d[qsl, :])
                    ot = att.tile([128, D], f32, tag="ot", bufs=3)
                    for j2 in range(D // 512):
                        pa = attn_ps.tile([128, 512], f32, tag="attn")
                        for j in range(SC):
                            nc.tensor.matmul(
                                pa,
                                lhsT=PT_sb[:, j, :],
                                rhs=v_sb[:, j, j2 * 512 : (j2 + 1) * 512],
                                start=(j == 0),
                                stop=(j == SC - 1),
                            )
                        # out = attn * (1/den) + residual
                        nc.vector.scalar_tensor_tensor(
                            out=ot[:, j2 * 512 : (j2 + 1) * 512],
                            in0=pa,
                            scalar=recip,
                            in1=xq_sb[:, j2 * 512 : (j2 + 1) * 512],
                            op0=Alu.mult,
                            op1=Alu.add,
                        )
                    nc.sync.dma_start(out=out_d[qsl, :], in_=ot)

    nc.compile()
    return nc


def _get_nc():
    if "nc" not in _cache:
        _cache["nc"] = _build()
    return _cache["nc"]


def kernel(embedded, Wq, bq, Wk, bk, Wv, bv):
    import ml_dtypes

    from concourse.bass_utils import run_bass_kernel_spmd

    bf16 = ml_dtypes.bfloat16
    x = np.ascontiguousarray(np.asarray(embedded, dtype=np.float32))
    Wq = np.asarray(Wq, dtype=np.float32)
    Wk = np.asarray(Wk, dtype=np.float32)
    Wv = np.asarray(Wv, dtype=np.float32)
    bq = np.ascontiguousarray(np.asarray(bq, dtype=np.float32))
    bk = np.ascontiguousarray(np.asarray(bk, dtype=np.float32))
    bv = np.ascontiguousarray(np.asarray(bv, dtype=np.float32))

    wqT = np.ascontiguousarray(Wq.T).astype(bf16)
    wkT = np.ascontiguousarray(Wk.T).astype(bf16)
    wvT = np.ascontiguousarray(Wv.T).astype(bf16)
    xT = [np.ascontiguousarray(x[b].T).astype(bf16) for b in range(B)]

    in_maps = []
    for c in range(NCORES):
        b, h = c // 2, c % 2
        qs = slice(h * SQ, (h + 1) * SQ)
        in_maps.append(
            {
                "xT": xT[b],
                "xqT": np.ascontiguousarray(xT[b][:, qs]),
                "xq": np.ascontiguousarray(x[b, qs, :] + bv),
                "wqT": wqT,
                "wkT": wkT,
                "wvT": wvT,
                "bq": bq,
            }
        )

    _cache["in_maps"] = in_maps
    nc = _get_nc()
    res = run_bass_kernel_spmd(nc, in_maps, core_ids=list(range(NCORES)))
    out = np.empty((B, S, D), dtype=np.float32)
    for c in range(NCORES):
        b, h = c // 2, c % 2
        out[b, h * SQ : (h + 1) * SQ, :] = res.results[c]["out"]
    return out



# revision 8
# speedup vs baseline: 1.3782x; 1.3782x over previous
"""Distributed single-head attention block for trn2 (8 NeuronCores), fp8.

reference:
    q = x @ Wq.T + bq ; k = x @ Wk.T + bk ; v = x @ Wv.T + bv
    out = x + softmax(q @ k.T / sqrt(D)) @ v       x: [4, 2048, 1024]

Sharding: 8 cores = 4 batches x 2 row-halves. Core c owns batch c//2 and
rows [h*1024, (h+1)*1024) with h = c%2. Each core projects Q/K/V for its
own rows only; K and V halves are exchanged via pairwise AllGather (the
own half takes the same DRAM roundtrip so the SPMD graph is uniform).

All five matmuls run fp8e4 with DoubleRow perf mode (2x PE throughput):
weights are pre-scaled by 32 on the host so W*32 ~ N(0,1) sits in the
fp8 normal range; the 32*32=1024 factor on scores folds into the exp
scale, and the 32 on V folds into the softmax reciprocal via a
32-valued ones-vector in the denominator matmul.

Scores are computed TRANSPOSED (scoresT[k,q] = K@Q^T) so the exp output
is already P^T, the lhsT the attention matmul needs — no PE transposes.
Softmax denominators (a partition-dim sum of P^T) come from a tiny
ones-vector matmul accumulated alongside attention. exp uses a -2 bias
(cancels between numerator and denominator) to keep P below fp8e4's
240 max; K's bias cancels in softmax; V's bias is folded into the
residual on the host (attention weights sum to 1).
"""

import numpy as np

B, S, D = 4, 2048, 1024
SQ = S // 2  # rows owned per core
NCORES = 8
DC = D // 128  # contraction chunks over embed
EC = D // 128  # output embed chunks
SC = S // 128  # key chunks
QT = SQ // 128  # query tiles per core
NPAIR = DC // 2  # DoubleRow pairs per 1024-deep contraction

WSCALE = 16.0  # host pre-scale on Wq/Wk/Wv (and bq); keeps |q,k,v| < 128
SCORE_SCALE = 1.0 / (np.sqrt(D) * WSCALE * WSCALE)  # 1/8192
EXP_SHIFT = 4.0  # exp(s - 4): scores reach ~7.4, so max P ~ e^3.5 << 240

_cache = {}


def _build():
    import concourse.bass as bass
    import concourse.tile as tile
    from concourse import bacc, mybir

    f32 = mybir.dt.float32
    bf16 = mybir.dt.bfloat16
    f8 = mybir.dt.float8e4
    Alu = mybir.AluOpType
    Act = mybir.ActivationFunctionType
    DR = mybir.MatmulPerfMode.DoubleRow

    nc = bacc.Bacc(None, target_bir_lowering=False, debug=False)

    xqT_d = nc.declare_dram_parameter("xqT", [128, DC, SQ], f8, isOutput=False)
    wq_d = nc.declare_dram_parameter("wqT", [128, DC, D], f8, isOutput=False)
    wk_d = nc.declare_dram_parameter("wkT", [128, DC, D], f8, isOutput=False)
    wv_d = nc.declare_dram_parameter("wvT", [128, DC, D], f8, isOutput=False)
    bq_d = nc.declare_dram_parameter("bq", [D], f32, isOutput=False)
    xq_d = nc.declare_dram_parameter("xq", [128, QT, D], f32, isOutput=False)
    out_d = nc.declare_dram_parameter("out", [SQ, D], f32, isOutput=True)

    kx_in = nc.dram_tensor("kx_in", [EC, 128, SQ], f8)
    kx_out = nc.dram_tensor("kx_out", [2, EC, 128, SQ], f8)
    vx_in = nc.dram_tensor("vx_in", [QT, 128, D], f8)
    vx_out = nc.dram_tensor("vx_out", [2, QT, 128, D], f8)
    GROUPS = [[0, 1], [2, 3], [4, 5], [6, 7]]

    with tile.TileContext(nc) as tc:
        with tc.tile_pool(name="pers", bufs=1) as pers:
            qT8 = pers.tile([128, EC, SQ], f8, tag="qT8")
            kT8 = pers.tile([128, EC, S], f8, tag="kT8")
            v8 = pers.tile([128, SC, D], f8, tag="v8")
            PT8 = pers.tile([128, SC, SQ], f8, tag="PT8")
            ones2 = pers.tile([128, 2, 1], f8, tag="ones2")
            bq_sb = pers.tile([128, EC], f32, tag="bq")
            nbias = pers.tile([128, 1], f32, tag="nbias")
            nc.vector.memset(ones2, WSCALE)
            nc.vector.memset(nbias, -float(EXP_SHIFT))

            # PE warmup: dummy matmuls while the first input DMAs land, so
            # the HAM clock gate is already ramped when real work starts.
            warm_sb = pers.tile([128, 512], bf16, tag="warm")
            warm_dump = pers.tile([128, 512], f32, tag="warm_dump")
            nc.gpsimd.memset(warm_sb, 0.0)
            with tc.tile_pool(name="warm_ps", bufs=1, space="PSUM") as warm_ps:
                wps = warm_ps.tile([128, 512], f32, tag="wps")
                NWARM = 14
                for i in range(NWARM):
                    nc.tensor.matmul(
                        wps,
                        lhsT=warm_sb[:, 0:128],
                        rhs=warm_sb,
                        start=(i == 0),
                        stop=(i == NWARM - 1),
                    )
                nc.vector.tensor_copy(out=warm_dump, in_=wps)

            bq_ap = bq_d.ap()
            nc.scalar.dma_start(
                out=bq_sb,
                in_=bass.AP(tensor=bq_ap.tensor, offset=0, ap=[[1, 128], [128, EC]]),
            )

            with (
                tc.tile_pool(name="ld", bufs=1) as ld,
                tc.tile_pool(name="proj_ps", bufs=4, space="PSUM") as proj_ps,
            ):
                xqT_sb = ld.tile([128, DC, SQ], f8, tag="xqT")
                wk_sb = ld.tile([128, DC, D], f8, tag="wk")
                wv_sb = ld.tile([128, DC, D], f8, tag="wv")
                wq_sb = ld.tile([128, DC, D], f8, tag="wq")
                xq_sb = ld.tile([128, QT, D], f32, tag="xq")
                kT_own = ld.tile([128, EC, SQ], f8, tag="kT_own")
                v_own = ld.tile([128, QT, D], f8, tag="v_own")

                # K-proj inputs first (they gate the earliest AllGather),
                # spread across queues so the two 1MB critical loads run
                # in parallel.
                nc.sync.dma_start(out=xqT_sb, in_=xqT_d.ap())
                nc.scalar.dma_start(out=wk_sb, in_=wk_d.ap())
                nc.gpsimd.dma_start(out=wv_sb, in_=wv_d.ap())
                nc.gpsimd.dma_start(out=wq_sb, in_=wq_d.ap())
                nc.sync.dma_start(out=xq_sb, in_=xq_d.ap())

                # kT_own[e, s_own] = sum_d (32*Wk)[e, d] * x[s_own, d]
                for ec in range(EC):
                    for sg in range(SQ // 512):
                        ps = proj_ps.tile([128, 512], f32, tag="ps")
                        for i in range(NPAIR):
                            nc.tensor.matmul(
                                ps,
                                lhsT=wk_sb[:, 2 * i : 2 * i + 2, ec * 128 : (ec + 1) * 128],
                                rhs=xqT_sb[:, 2 * i : 2 * i + 2, sg * 512 : (sg + 1) * 512],
                                start=(i == 0),
                                stop=(i == NPAIR - 1),
                                perf_mode=DR,
                            )
                        nc.scalar.activation(
                            out=kT_own[:, ec, sg * 512 : (sg + 1) * 512],
                            in_=ps,
                            func=Act.Copy,
                        )
                kx_ap = kx_in.ap()
                nc.sync.dma_start(
                    out=bass.AP(
                        tensor=kx_ap.tensor,
                        offset=0,
                        ap=[[SQ, 128], [128 * SQ, EC], [1, SQ]],
                    ),
                    in_=kT_own,
                )
                nc.gpsimd.collective_compute(
                    "AllGather",
                    mybir.AluOpType.bypass,
                    replica_groups=GROUPS,
                    ins=[kx_in.ap().opt()],
                    outs=[kx_out.ap().opt()],
                )

                # v_own[s_own, e] = sum_d x[s_own, d] * (32*Wv)[e, d]
                for st in range(QT):
                    for eg in range(D // 512):
                        ps = proj_ps.tile([128, 512], f32, tag="ps")
                        for i in range(NPAIR):
                            nc.tensor.matmul(
                                ps,
                                lhsT=xqT_sb[:, 2 * i : 2 * i + 2, st * 128 : (st + 1) * 128],
                                rhs=wv_sb[:, 2 * i : 2 * i + 2, eg * 512 : (eg + 1) * 512],
                                start=(i == 0),
                                stop=(i == NPAIR - 1),
                                perf_mode=DR,
                            )
                        nc.scalar.activation(
                            out=v_own[:, st, eg * 512 : (eg + 1) * 512],
                            in_=ps,
                            func=Act.Copy,
                        )
                vx_ap = vx_in.ap()
                nc.sync.dma_start(
                    out=bass.AP(
                        tensor=vx_ap.tensor,
                        offset=0,
                        ap=[[D, 128], [128 * D, QT], [1, D]],
                    ),
                    in_=v_own,
                )
                nc.gpsimd.collective_compute(
                    "AllGather",
                    mybir.AluOpType.bypass,
                    replica_groups=GROUPS,
                    ins=[vx_in.ap().opt()],
                    outs=[vx_out.ap().opt()],
                )

                # qT8[e, q] = sum_d (32*Wq)[e, d] * x[q, d]  (+32*bq per e)
                for ec in range(EC):
                    for sg in range(SQ // 512):
                        ps = proj_ps.tile([128, 512], f32, tag="ps")
                        for i in range(NPAIR):
                            nc.tensor.matmul(
                                ps,
                                lhsT=wq_sb[:, 2 * i : 2 * i + 2, ec * 128 : (ec + 1) * 128],
                                rhs=xqT_sb[:, 2 * i : 2 * i + 2, sg * 512 : (sg + 1) * 512],
                                start=(i == 0),
                                stop=(i == NPAIR - 1),
                                perf_mode=DR,
                            )
                        nc.vector.tensor_scalar_add(
                            out=qT8[:, ec, sg * 512 : (sg + 1) * 512],
                            in0=ps,
                            scalar1=bq_sb[:, ec : ec + 1],
                        )

                # exchange readback: both slots (uniform SPMD graph; the own
                # slot roundtrips through DRAM with identical data)
                kxo_ap = kx_out.ap()
                for r in range(2):
                    nc.sync.dma_start(
                        out=kT8[:, :, r * SQ : (r + 1) * SQ],
                        in_=bass.AP(
                            tensor=kxo_ap.tensor,
                            offset=r * EC * 128 * SQ,
                            ap=[[SQ, 128], [128 * SQ, EC], [1, SQ]],
                        ),
                    )
                vxo_ap = vx_out.ap()
                for r in range(2):
                    nc.scalar.dma_start(
                        out=v8[:, r * QT : (r + 1) * QT, :],
                        in_=bass.AP(
                            tensor=vxo_ap.tensor,
                            offset=r * QT * 128 * D,
                            ap=[[D, 128], [128 * D, QT], [1, D]],
                        ),
                    )

            with (
                tc.tile_pool(name="att", bufs=3) as att,
                tc.tile_pool(name="small", bufs=2) as small,
                tc.tile_pool(name="score_ps", bufs=3, space="PSUM") as score_ps,
                tc.tile_pool(name="attn_ps", bufs=3, space="PSUM") as attn_ps,
                tc.tile_pool(name="den_ps", bufs=2, space="PSUM") as den_ps,
            ):
                # scoresT[k, q] = sum_e kT8[e, k] * qT8[e, q]; exp -> PT8
                for kc in range(SC):
                    for qg in range(SQ // 512):
                        ps = score_ps.tile([128, 512], f32, tag="score")
                        for i in range(EC // 2):
                            nc.tensor.matmul(
                                ps,
                                lhsT=kT8[:, 2 * i : 2 * i + 2, kc * 128 : (kc + 1) * 128],
                                rhs=qT8[:, 2 * i : 2 * i + 2, qg * 512 : (qg + 1) * 512],
                                start=(i == 0),
                                stop=(i == EC // 2 - 1),
                                perf_mode=DR,
                            )
                        nc.scalar.activation(
                            out=PT8[:, kc, qg * 512 : (qg + 1) * 512],
                            in_=ps,
                            func=Act.Exp,
                            scale=float(SCORE_SCALE),
                            bias=nbias,
                        )

                # attn + denominators + epilogue per q-tile
                for qt in range(QT):
                    qsl = slice(qt * 128, (qt + 1) * 128)
                    dn = den_ps.tile([128, 1], f32, tag="den")
                    for j in range(SC // 2):
                        nc.tensor.matmul(
                            dn,
                            lhsT=PT8[:, 2 * j : 2 * j + 2, qsl],
                            rhs=ones2,
                            start=(j == 0),
                            stop=(j == SC // 2 - 1),
                            perf_mode=DR,
                        )
                    recip = small.tile([128, 1], f32, tag="recip", bufs=4)
                    nc.vector.reciprocal(recip, dn)
                    ot = att.tile([128, D], f32, tag="ot")
                    for j2 in range(D // 512):
                        pa = attn_ps.tile([128, 512], f32, tag="attn")
                        for j in range(SC // 2):
                            nc.tensor.matmul(
                                pa,
                                lhsT=PT8[:, 2 * j : 2 * j + 2, qsl],
                                rhs=v8[:, 2 * j : 2 * j + 2, j2 * 512 : (j2 + 1) * 512],
                                start=(j == 0),
                                stop=(j == SC // 2 - 1),
                                perf_mode=DR,
                            )
                        # out = attn * (1/(32*den)) + residual
                        nc.vector.scalar_tensor_tensor(
                            out=ot[:, j2 * 512 : (j2 + 1) * 512],
                            in0=pa,
                            scalar=recip,
                            in1=xq_sb[:, qt, j2 * 512 : (j2 + 1) * 512],
                            op0=Alu.mult,
                            op1=Alu.add,
                        )
                    nc.sync.dma_start(out=out_d[qsl, :], in_=ot)

    nc.compile()
    return nc


def _get_nc():
    if "nc" not in _cache:
        _cache["nc"] = _build()
    return _cache["nc"]


def _swizzle(a, np_f8):
    """[D, N] -> [128, D//128, N] partition-major, cast to fp8."""
    d, n = a.shape
    return np.ascontiguousarray(
        a.reshape(d // 128, 128, n).transpose(1, 0, 2)
    ).astype(np_f8)


def kernel(embedded, Wq, bq, Wk, bk, Wv, bv):
    import ml_dtypes

    from concourse.bass_utils import run_bass_kernel_spmd

    f8 = ml_dtypes.float8_e4m3
    x = np.ascontiguousarray(np.asarray(embedded, dtype=np.float32))
    Wq = np.asarray(Wq, dtype=np.float32)
    Wk = np.asarray(Wk, dtype=np.float32)
    Wv = np.asarray(Wv, dtype=np.float32)
    bq = np.ascontiguousarray(np.asarray(bq, dtype=np.float32))
    bv = np.ascontiguousarray(np.asarray(bv, dtype=np.float32))

    wqT = _swizzle(np.ascontiguousarray(Wq.T) * WSCALE, f8)
    wkT = _swizzle(np.ascontiguousarray(Wk.T) * WSCALE, f8)
    wvT = _swizzle(np.ascontiguousarray(Wv.T) * WSCALE, f8)
    bq32 = np.ascontiguousarray(bq * WSCALE)

    in_maps = []
    for c in range(NCORES):
        b, h = c // 2, c % 2
        qs = slice(h * SQ, (h + 1) * SQ)
        xh = x[b, qs, :]  # [SQ, D]
        in_maps.append(
            {
                "xqT": _swizzle(np.ascontiguousarray(xh.T), f8),
                "xq": np.ascontiguousarray(
                    (xh + bv).reshape(QT, 128, D).transpose(1, 0, 2)
                ),
                "wqT": wqT,
                "wkT": wkT,
                "wvT": wvT,
                "bq": bq32,
            }
        )

    _cache["in_maps"] = in_maps
    nc = _get_nc()
    res = run_bass_kernel_spmd(nc, in_maps, core_ids=list(range(NCORES)))
    out = np.empty((B, S, D), dtype=np.float32)
    for c in range(NCORES):
        b, h = c // 2, c % 2
        out[b, h * SQ : (h + 1) * SQ, :] = res.results[c]["out"]
    return out


# revision 9
# speedup vs baseline: 1.4837x; 1.0766x over previous
"""Distributed single-head attention block for trn2 (8 NeuronCores), fp8.

reference:
    q = x @ Wq.T + bq ; k = x @ Wk.T + bk ; v = x @ Wv.T + bv
    out = x + softmax(q @ k.T / sqrt(D)) @ v       x: [4, 2048, 1024]

Sharding: 8 cores = 4 batches x 2 query-halves. Core c owns batch c//2 and
query rows [h*1024, (h+1)*1024) with h = c%2. Each core recomputes K for
its whole batch (duplicated across the pair — cheaper than the serialized
CC-stream hop a K-exchange costs); V is projected for the own half only
and exchanged via pairwise AllGather, which hides under scoresT. A dummy
128B AllGather issued at kernel start prefetches the ~20us CC rendezvous
barrier so the real exchange starts immediately.

All matmuls run fp8e4 with DoubleRow perf mode (2x PE throughput):
weights are pre-scaled by 16 on the host so W*16 sits in the fp8 normal
range with |q,k,v| < 128 (fp8e4 max 240); the 16*16 factor on scores
folds into the exp scale, and the 16 on V folds into the softmax
reciprocal via a 16-valued ones-vector in the denominator matmul.

Scores are computed TRANSPOSED (scoresT[k,q] = K@Q^T) so the exp output
is already P^T, the lhsT the attention matmul needs — no PE transposes.
Softmax denominators (a partition-dim sum of P^T) come from a tiny
ones-vector matmul. exp uses a -4 bias (cancels between numerator and
denominator; scores reach ~7.4) to keep P far below fp8e4's 240 max.
K's bias cancels in softmax; V's bias is folded into the residual on
the host (attention weights sum to 1).
"""

import numpy as np

B, S, D = 4, 2048, 1024
SQ = S // 2  # query rows owned per core
NCORES = 8
DC = D // 128  # contraction chunks over embed
EC = D // 128  # output embed chunks
SC = S // 128  # key chunks
QT = SQ // 128  # query tiles per core
NPAIR = DC // 2  # DoubleRow pairs per 1024-deep contraction

WSCALE = 16.0  # host pre-scale on Wq/Wk/Wv (and bq); keeps |q,k,v| < 128
SCORE_SCALE = 1.0 / (np.sqrt(D) * WSCALE * WSCALE)  # 1/8192
EXP_SHIFT = 4.0  # exp(s - 4): scores reach ~7.4, so max P ~ e^3.5 << 240

_cache = {}


def _build():
    import concourse.bass as bass
    import concourse.tile as tile
    from concourse import bacc, mybir

    f32 = mybir.dt.float32
    bf16 = mybir.dt.bfloat16
    f8 = mybir.dt.float8e4
    Alu = mybir.AluOpType
    Act = mybir.ActivationFunctionType
    DR = mybir.MatmulPerfMode.DoubleRow

    nc = bacc.Bacc(None, target_bir_lowering=False, debug=False)

    xT_d = nc.declare_dram_parameter("xT", [128, DC, S], f8, isOutput=False)
    xqT_d = nc.declare_dram_parameter("xqT", [128, DC, SQ], f8, isOutput=False)
    wq_d = nc.declare_dram_parameter("wqT", [128, DC, D], f8, isOutput=False)
    wk_d = nc.declare_dram_parameter("wkT", [128, DC, D], f8, isOutput=False)
    wv_d = nc.declare_dram_parameter("wvT", [128, DC, D], f8, isOutput=False)
    bq_d = nc.declare_dram_parameter("bq", [D], f32, isOutput=False)
    xq_d = nc.declare_dram_parameter("xq", [128, QT, D], f32, isOutput=False)
    out_d = nc.declare_dram_parameter("out", [SQ, D], f32, isOutput=True)

    vx_in = nc.dram_tensor("vx_in", [QT, 128, D], f8)
    vx_out = nc.dram_tensor("vx_out", [2, QT, 128, D], f8)
    dumb_in = nc.dram_tensor("dumb_in", [128], f8)
    dumb_out = nc.dram_tensor("dumb_out", [2, 128], f8)
    GROUPS = [[0, 1], [2, 3], [4, 5], [6, 7]]

    with tile.TileContext(nc) as tc:
        with tc.tile_pool(name="pers", bufs=1) as pers:
            # Barrier prefetch: the first collective on the CC stream pays a
            # ~20us all-core rendezvous. Fire it on a 128B dummy now so the
            # real V exchange later starts transferring immediately.
            nc.gpsimd.collective_compute(
                "AllGather",
                mybir.AluOpType.bypass,
                replica_groups=GROUPS,
                ins=[dumb_in.ap().opt()],
                outs=[dumb_out.ap().opt()],
            )

            qT8 = pers.tile([128, EC, SQ], f8, tag="qT8")
            kT8 = pers.tile([128, EC, S], f8, tag="kT8")
            v8 = pers.tile([128, SC, D], f8, tag="v8")
            PT8 = pers.tile([128, SC, SQ], f8, tag="PT8")
            ones2 = pers.tile([128, 2, 1], f8, tag="ones2")
            bq_sb = pers.tile([128, EC], f32, tag="bq")
            nbias = pers.tile([128, 1], f32, tag="nbias")
            nc.vector.memset(ones2, WSCALE)
            nc.vector.memset(nbias, -float(EXP_SHIFT))

            # PE warmup: dummy matmuls while the first input DMAs land, so
            # the HAM clock gate is already ramped when real work starts.
            warm_sb = pers.tile([128, 512], bf16, tag="warm")
            warm_dump = pers.tile([128, 512], f32, tag="warm_dump")
            nc.vector.memset(warm_sb, 0.0)
            with tc.tile_pool(name="warm_ps", bufs=1, space="PSUM") as warm_ps:
                wps = warm_ps.tile([128, 512], f32, tag="wps")
                NWARM = 14
                for i in range(NWARM):
                    nc.tensor.matmul(
                        wps,
                        lhsT=warm_sb[:, 0:128],
                        rhs=warm_sb,
                        start=(i == 0),
                        stop=(i == NWARM - 1),
                    )
                nc.vector.tensor_copy(out=warm_dump, in_=wps)

            bq_ap = bq_d.ap()
            nc.scalar.dma_start(
                out=bq_sb,
                in_=bass.AP(tensor=bq_ap.tensor, offset=0, ap=[[1, 128], [128, EC]]),
            )

            with (
                tc.tile_pool(name="ld", bufs=1) as ld,
                tc.tile_pool(name="proj_ps", bufs=4, space="PSUM") as proj_ps,
            ):
                xT_sb = ld.tile([128, DC, S], f8, tag="xT")
                xqT_sb = ld.tile([128, DC, SQ], f8, tag="xqT")
                wk_sb = ld.tile([128, DC, D], f8, tag="wk")
                wv_sb = ld.tile([128, DC, D], f8, tag="wv")
                wq_sb = ld.tile([128, DC, D], f8, tag="wq")
                xq_sb = ld.tile([128, QT, D], f32, tag="xq")
                v_own = ld.tile([128, QT, D], f8, tag="v_own")

                # Queue balance: sync carries only the K-proj-critical xT,
                # then V staging; scalar carries wk/xqT then V readbacks;
                # gpsimd carries the rest (and the collective triggers).
                nc.sync.dma_start(out=xT_sb, in_=xT_d.ap())
                nc.scalar.dma_start(out=wk_sb, in_=wk_d.ap())
                nc.scalar.dma_start(out=xqT_sb, in_=xqT_d.ap())
                nc.gpsimd.dma_start(out=wv_sb, in_=wv_d.ap())
                nc.gpsimd.dma_start(out=wq_sb, in_=wq_d.ap())
                nc.gpsimd.dma_start(out=xq_sb, in_=xq_d.ap())

                # kT8[e, s] = sum_d (16*Wk)[e, d] * x[s, d]  (full batch)
                for ec in range(EC):
                    for sg in range(S // 512):
                        ps = proj_ps.tile([128, 512], f32, tag="ps")
                        for i in range(NPAIR):
                            nc.tensor.matmul(
                                ps,
                                lhsT=wk_sb[:, 2 * i : 2 * i + 2, ec * 128 : (ec + 1) * 128],
                                rhs=xT_sb[:, 2 * i : 2 * i + 2, sg * 512 : (sg + 1) * 512],
                                start=(i == 0),
                                stop=(i == NPAIR - 1),
                                perf_mode=DR,
                            )
                        nc.scalar.activation(
                            out=kT8[:, ec, sg * 512 : (sg + 1) * 512],
                            in_=ps,
                            func=Act.Copy,
                        )

                # v_own[s_own, e] = sum_d x[s_own, d] * (16*Wv)[e, d]
                for st in range(QT):
                    for eg in range(D // 512):
                        ps = proj_ps.tile([128, 512], f32, tag="ps")
                        for i in range(NPAIR):
                            nc.tensor.matmul(
                                ps,
                                lhsT=xqT_sb[:, 2 * i : 2 * i + 2, st * 128 : (st + 1) * 128],
                                rhs=wv_sb[:, 2 * i : 2 * i + 2, eg * 512 : (eg + 1) * 512],
                                start=(i == 0),
                                stop=(i == NPAIR - 1),
                                perf_mode=DR,
                            )
                        nc.vector.tensor_copy(
                            out=v_own[:, st, eg * 512 : (eg + 1) * 512],
                            in_=ps,
                        )
                vx_ap = vx_in.ap()
                nc.sync.dma_start(
                    out=bass.AP(
                        tensor=vx_ap.tensor,
                        offset=0,
                        ap=[[D, 128], [128 * D, QT], [1, D]],
                    ),
                    in_=v_own,
                )
                nc.gpsimd.collective_compute(
                    "AllGather",
                    mybir.AluOpType.bypass,
                    replica_groups=GROUPS,
                    ins=[vx_in.ap().opt()],
                    outs=[vx_out.ap().opt()],
                )

                # qT8[e, q] = sum_d (16*Wq)[e, d] * x[q, d]  (+16*bq per e)
                for ec in range(EC):
                    for sg in range(SQ // 512):
                        ps = proj_ps.tile([128, 512], f32, tag="ps")
                        for i in range(NPAIR):
                            nc.tensor.matmul(
                                ps,
                                lhsT=wq_sb[:, 2 * i : 2 * i + 2, ec * 128 : (ec + 1) * 128],
                                rhs=xqT_sb[:, 2 * i : 2 * i + 2, sg * 512 : (sg + 1) * 512],
                                start=(i == 0),
                                stop=(i == NPAIR - 1),
                                perf_mode=DR,
                            )
                        nc.vector.tensor_scalar_add(
                            out=qT8[:, ec, sg * 512 : (sg + 1) * 512],
                            in0=ps,
                            scalar1=bq_sb[:, ec : ec + 1],
                        )

                # V readback: both slots (uniform SPMD graph; the own slot
                # roundtrips through DRAM with identical data)
                vxo_ap = vx_out.ap()
                for r in range(2):
                    nc.scalar.dma_start(
                        out=v8[:, r * QT : (r + 1) * QT, :],
                        in_=bass.AP(
                            tensor=vxo_ap.tensor,
                            offset=r * QT * 128 * D,
                            ap=[[D, 128], [128 * D, QT], [1, D]],
                        ),
                    )

            with (
                tc.tile_pool(name="att", bufs=3) as att,
                tc.tile_pool(name="small", bufs=2) as small,
                tc.tile_pool(name="score_ps", bufs=3, space="PSUM") as score_ps,
                tc.tile_pool(name="attn_ps", bufs=3, space="PSUM") as attn_ps,
                tc.tile_pool(name="den_ps", bufs=2, space="PSUM") as den_ps,
            ):
                # scoresT[k, q] = sum_e kT8[e, k] * qT8[e, q]; exp -> PT8
                for kc in range(SC):
                    for qg in range(SQ // 512):
                        ps = score_ps.tile([128, 512], f32, tag="score")
                        for i in range(EC // 2):
                            nc.tensor.matmul(
                                ps,
                                lhsT=kT8[:, 2 * i : 2 * i + 2, kc * 128 : (kc + 1) * 128],
                                rhs=qT8[:, 2 * i : 2 * i + 2, qg * 512 : (qg + 1) * 512],
                                start=(i == 0),
                                stop=(i == EC // 2 - 1),
                                perf_mode=DR,
                            )
                        nc.scalar.activation(
                            out=PT8[:, kc, qg * 512 : (qg + 1) * 512],
                            in_=ps,
                            func=Act.Exp,
                            scale=float(SCORE_SCALE),
                            bias=nbias,
                        )

                # attn + denominators + epilogue per q-tile
                for qt in range(QT):
                    qsl = slice(qt * 128, (qt + 1) * 128)
                    dn = den_ps.tile([128, 1], f32, tag="den")
                    for j in range(SC // 2):
                        nc.tensor.matmul(
                            dn,
                            lhsT=PT8[:, 2 * j : 2 * j + 2, qsl],
                            rhs=ones2,
                            start=(j == 0),
                            stop=(j == SC // 2 - 1),
                            perf_mode=DR,
                        )
                    recip = small.tile([128, 1], f32, tag="recip", bufs=4)
                    nc.vector.reciprocal(recip, dn)
                    ot = att.tile([128, D], f32, tag="ot")
                    for j2 in range(D // 512):
                        pa = attn_ps.tile([128, 512], f32, tag="attn")
                        for j in range(SC // 2):
                            nc.tensor.matmul(
                                pa,
                                lhsT=PT8[:, 2 * j : 2 * j + 2, qsl],
                                rhs=v8[:, 2 * j : 2 * j + 2, j2 * 512 : (j2 + 1) * 512],
                                start=(j == 0),
                                stop=(j == SC // 2 - 1),
                                perf_mode=DR,
                            )
                        # out = attn * (1/(16*den)) + residual
                        nc.vector.scalar_tensor_tensor(
                            out=ot[:, j2 * 512 : (j2 + 1) * 512],
                            in0=pa,
                            scalar=recip,
                            in1=xq_sb[:, qt, j2 * 512 : (j2 + 1) * 512],
                            op0=Alu.mult,
                            op1=Alu.add,
                        )
                        nc.sync.dma_start(
                            out=out_d[qsl, j2 * 512 : (j2 + 1) * 512],
                            in_=ot[:, j2 * 512 : (j2 + 1) * 512],
                        )

    nc.compile()
    return nc


def _get_nc():
    if "nc" not in _cache:
        _cache["nc"] = _build()
    return _cache["nc"]


def _swizzle(a, np_f8):
    """[D, N] -> [128, D//128, N] partition-major, cast to fp8."""
    d, n = a.shape
    return np.ascontiguousarray(
        a.reshape(d // 128, 128, n).transpose(1, 0, 2)
    ).astype(np_f8)


def kernel(embedded, Wq, bq, Wk, bk, Wv, bv):
    import ml_dtypes

    from concourse.bass_utils import run_bass_kernel_spmd

    f8 = ml_dtypes.float8_e4m3
    x = np.ascontiguousarray(np.asarray(embedded, dtype=np.float32))
    Wq = np.asarray(Wq, dtype=np.float32)
    Wk = np.asarray(Wk, dtype=np.float32)
    Wv = np.asarray(Wv, dtype=np.float32)
    bq = np.ascontiguousarray(np.asarray(bq, dtype=np.float32))
    bv = np.ascontiguousarray(np.asarray(bv, dtype=np.float32))

    wqT = _swizzle(np.ascontiguousarray(Wq.T) * WSCALE, f8)
    wkT = _swizzle(np.ascontiguousarray(Wk.T) * WSCALE, f8)
    wvT = _swizzle(np.ascontiguousarray(Wv.T) * WSCALE, f8)
    bq16 = np.ascontiguousarray(bq * WSCALE)

    xT8 = [_swizzle(np.ascontiguousarray(x[b].T), f8) for b in range(B)]

    in_maps = []
    for c in range(NCORES):
        b, h = c // 2, c % 2
        qs = slice(h * SQ, (h + 1) * SQ)
        xh = x[b, qs, :]  # [SQ, D]
        in_maps.append(
            {
                "xT": xT8[b],
                "xqT": np.ascontiguousarray(xT8[b][:, :, qs]),
                "xq": np.ascontiguousarray(
                    (xh + bv).reshape(QT, 128, D).transpose(1, 0, 2)
                ),
                "wqT": wqT,
                "wkT": wkT,
                "wvT": wvT,
                "bq": bq16,
            }
        )

    _cache["in_maps"] = in_maps
    nc = _get_nc()
    res = run_bass_kernel_spmd(nc, in_maps, core_ids=list(range(NCORES)))
    out = np.empty((B, S, D), dtype=np.float32)
    for c in range(NCORES):
        b, h = c // 2, c % 2
        out[b, h * SQ : (h + 1) * SQ, :] = res.results[c]["out"]
    return out
